# revision 6
# baseline (speedup 1.0000x reference)
import sys

sys.path.insert(0, "/opt/trn_rl_repo")

import numpy as np
from concourse import bass, bacc, tile, mybir
from concourse.bass_utils import run_bass_kernel_spmd

WAY = 5
ROI = 7
C = 256
REP = 1024
BOX = 4096
HID = 64
BN_EPS = 1e-5
NCORES = 8
B = BOX // NCORES  # 512 boxes per core
HW = ROI * ROI  # 49
FEAT = C * HW  # 12544
NOUT = HID + WAY  # 69 (flat eff + score eff columns)
USE_LRELU = True

F32 = mybir.dt.float32
AF = mybir.ActivationFunctionType
ALU = mybir.AluOpType

_compiled = {}
TRACE = False
LAST = None
LAST_INMAPS = None
LAST_NC = None


def _leaky_np(v):
    return np.where(v >= 0, v, 0.01 * v).astype(np.float32)


def _build_program(consts):
    """Build the SPMD bass program. consts: dict of host-derived floats."""
    nc = bacc.Bacc("TRN2", debug=False, target_bir_lowering=False,
                   num_devices=NCORES)

    qT = nc.dram_tensor("qT", [FEAT, B], F32, kind="ExternalInput").ap()
    xT = nc.dram_tensor("xT", [REP, B], F32, kind="ExternalInput").ap()
    weffsw = nc.dram_tensor("weffsw", [128, 98 * NOUT], F32, kind="ExternalInput").ap()
    convwt = nc.dram_tensor("convwt", [128, 2 * HID], F32, kind="ExternalInput").ap()
    wcatsw = nc.dram_tensor("wcatsw", [128, 8 * 25], F32, kind="ExternalInput").ap()
    beff = nc.dram_tensor("beff", [NOUT, 1], F32, kind="ExternalInput").ap()
    bna = nc.dram_tensor("bna", [HID, 1], F32, kind="ExternalInput").ap()
    bnd = nc.dram_tensor("bnd", [HID, 1], F32, kind="ExternalInput").ap()
    bcat = nc.dram_tensor("bcat", [25, 1], F32, kind="ExternalInput").ap()
    decw = nc.dram_tensor("decw", [6, 6], F32, kind="ExternalInput").ap()
    decb = nc.dram_tensor("decb", [6, 1], F32, kind="ExternalInput").ap()
    # square-branch per-partition biases: -sqrt(w)*support_mean
    sqb = nc.dram_tensor("sqb", [WAY, 3], F32, kind="ExternalInput").ap()
    rmask = nc.dram_tensor("rmask", [NOUT, 2], F32, kind="ExternalInput").ap()
    sel = nc.dram_tensor("sel", [2, 10], F32, kind="ExternalInput").ap()

    scoresT = nc.dram_tensor("scoresT", [6, B], F32, kind="ExternalOutput").ap()
    bboxT = nc.dram_tensor("bboxT", [24, B], F32, kind="ExternalOutput").ap()

    s_f, s_c, s_s = consts["s_f"], consts["s_c"], consts["s_s"]
    es = consts["es"]

    with tile.TileContext(nc) as tc:
        with (
            tc.tile_pool(name="wp", bufs=1) as wp,
            tc.tile_pool(name="qp", bufs=8) as qp,
            tc.tile_pool(name="sp", bufs=2) as sp,
            tc.tile_pool(name="ap", bufs=1) as app,
            tc.tile_pool(name="ph", bufs=1, space="PSUM") as ph,
            tc.tile_pool(name="py", bufs=3, space="PSUM") as py,
            tc.tile_pool(name="pm", bufs=2, space="PSUM") as pm,
        ):
            w_eff = wp.tile([128, 98 * NOUT], F32)
            nc.sync.dma_start(w_eff[:], weffsw[:])
            w_conv = wp.tile([128, 2 * HID], F32)
            nc.sync.dma_start(w_conv[:], convwt[:])
            w_cat = wp.tile([128, 8 * 25], F32)
            nc.sync.dma_start(w_cat[:], wcatsw[:])
            t_beff = wp.tile([NOUT, 1], F32)
            nc.sync.dma_start(t_beff[:], beff[:])
            t_bna = wp.tile([HID, 1], F32)
            nc.sync.dma_start(t_bna[:], bna[:])
            t_bnd = wp.tile([HID, 1], F32)
            nc.sync.dma_start(t_bnd[:], bnd[:])
            t_bcat = wp.tile([25, 1], F32)
            nc.sync.dma_start(t_bcat[:], bcat[:])
            t_decw = wp.tile([6, 6], F32)
            nc.sync.dma_start(t_decw[:], decw[:])
            t_decb = wp.tile([6, 1], F32)
            nc.sync.dma_start(t_decb[:], decb[:])
            t_sqb = wp.tile([WAY, 3], F32)
            nc.sync.dma_start(t_sqb[:], sqb[:])
            t_rmask = wp.tile([NOUT, 2], F32)
            nc.sync.dma_start(t_rmask[:], rmask[:])
            t_sel = wp.tile([2, 10], F32)
            nc.sync.dma_start(t_sel[:], sel[:])
            ones = wp.tile([64, 8], F32)
            nc.any.memset(ones[:], 1.0)

            def leaky(out_ap, in_ap, bias, scale):
                if USE_LRELU:
                    nc.scalar.activation(out_ap, in_ap, AF.Lrelu,
                                         bias=bias, scale=scale, alpha=0.01)
                else:
                    nc.scalar.activation(out_ap, in_ap, AF.Identity,
                                         bias=bias, scale=scale)
                    tmp = sp.tile(list(out_ap.shape), F32, tag="lk")
                    nc.vector.tensor_scalar(tmp[:], out_ap, 0.01, None,
                                            ALU.mult)
                    nc.vector.tensor_tensor(out_ap, out_ap, tmp[:], ALU.max)

            # ---- main loop over spatial positions: flat + conv matmuls ----
            qTr = qT.rearrange("(c hw) b -> c hw b", hw=HW)
            ps_h = ph.tile([NOUT, B], F32)
            acc = app.tile([HID, B], F32)
            for hw in range(HW):
                ps_y = py.tile([HID, B], F32, tag="psy")
                for cc in range(2):
                    idx = hw * 2 + cc
                    t = qp.tile([128, B], F32, tag="q")
                    nc.sync.dma_start(t[:], qTr[cc * 128:(cc + 1) * 128, hw, :])
                    nc.tensor.matmul(ps_h[:], w_eff[:, idx * NOUT:(idx + 1) * NOUT],
                                     t[:], start=(idx == 0), stop=(idx == 97))
                    nc.tensor.matmul(ps_y[:], w_conv[:, cc * HID:(cc + 1) * HID],
                                     t[:], start=(cc == 0), stop=(cc == 1))
                z = sp.tile([HID, B], F32, tag="z")
                leaky(z[:], ps_y[:], t_bnd[:], t_bna[:])
                if hw == 0:
                    nc.vector.tensor_copy(acc[:], z[:])
                else:
                    nc.vector.tensor_tensor(acc[:], acc[:], z[:], ALU.add)

            # ---- qc sum over (channel, hw) ----
            ps_qc = pm.tile([1, B], F32, tag="sm")
            nc.tensor.matmul(ps_qc[:], ones[:, 0:1], acc[:], start=True, stop=True)
            qc_s = sp.tile([1, B], F32, tag="qcs")
            nc.vector.tensor_copy(qc_s[:], ps_qc[:])

            # ---- flat/score activations + their sums (one masked matmul) ----
            qfs = sp.tile([NOUT, B], F32, tag="qfs")
            leaky(qfs[:], ps_h[:], t_beff[:], 1.0)
            ps_fs = pm.tile([2, B], F32, tag="sm")
            nc.tensor.matmul(ps_fs[:], t_rmask[:], qfs[:], start=True, stop=True)
            fs_s = sp.tile([2, B], F32, tag="fss")
            nc.vector.tensor_copy(fs_s[:], ps_fs[:])

            # ---- bbox / bg branch ----
            ps_b = pm.tile([25, B], F32, tag="sm")
            for kk in range(8):
                xt = qp.tile([128, B], F32, tag="q")
                nc.sync.dma_start(xt[:], xT[kk * 128:(kk + 1) * 128, :])
                nc.tensor.matmul(ps_b[:], w_cat[:, kk * 25:(kk + 1) * 25],
                                 xt[:], start=(kk == 0), stop=(kk == 7))
            bbt = sp.tile([25, B], F32, tag="bbt")
            nc.scalar.activation(bbt[:], ps_b[:], AF.Identity,
                                 bias=t_bcat[:])
            nc.sync.dma_start(bboxT[:], bbt[1:25, :])

            # ---- broadcast means to 5 partitions and square ----
            ps_bf = pm.tile([WAY, B], F32, tag="sm")
            nc.tensor.matmul(ps_bf[:], t_sel[:, 0:5], fs_s[:], start=True, stop=True)
            sq_f = sp.tile([WAY, B], F32, tag="sqf")
            nc.scalar.activation(sq_f[:], ps_bf[:], AF.Square,
                                 bias=t_sqb[:, 0:1], scale=s_f)
            ps_bc = pm.tile([WAY, B], F32, tag="sm")
            nc.tensor.matmul(ps_bc[:], ones[0:1, 0:5], qc_s[:], start=True, stop=True)
            sq_c = sp.tile([WAY, B], F32, tag="sqc")
            nc.scalar.activation(sq_c[:], ps_bc[:], AF.Square,
                                 bias=t_sqb[:, 1:2], scale=s_c)
            ps_bs = pm.tile([WAY, B], F32, tag="sm")
            nc.tensor.matmul(ps_bs[:], t_sel[:, 5:10], fs_s[:], start=True, stop=True)
            sq_s = sp.tile([WAY, B], F32, tag="sqs")
            nc.scalar.activation(sq_s[:], ps_bs[:], AF.Square,
                                 bias=t_sqb[:, 2:3], scale=s_s)

            # ---- distance^T assembly [6, B] ----
            D = sp.tile([6, B], F32, tag="D")
            nc.vector.tensor_tensor(D[0:5, :], sq_f[:], sq_c[:], ALU.add)
            nc.vector.tensor_tensor(D[0:5, :], D[0:5, :], sq_s[:], ALU.add)
            nc.sync.dma_start(D[5:6, :], bbt[0:1, :])

            # ---- decoder: leaky(dist @ dec_W + dec_b) -> sigmoid -> affine ----
            ps_d = pm.tile([6, B], F32, tag="sm")
            nc.tensor.matmul(ps_d[:], t_decw[:], D[:], start=True, stop=True)
            d1 = sp.tile([6, B], F32, tag="d1")
            leaky(d1[:], ps_d[:], t_decb[:], 1.0)
            d2 = sp.tile([6, B], F32, tag="d2")
            nc.scalar.activation(d2[:], d1[:], AF.Sigmoid)
            d3 = sp.tile([6, B], F32, tag="d3")
            nc.scalar.activation(d3[:], d2[:], AF.Copy, bias=es, scale=-2.0 * es)
            nc.sync.dma_start(scoresT[:], d3[:])

    nc.compile()
    return nc


def kernel(support, query, x, bbox_W, bbox_b, conv_W, conv_b, bn_gamma, bn_beta,
           flat_W1, flat_b1, flat_W2, flat_b2, score_W1, score_b1, score_W2,
           score_b2, bg_W, bg_b, dec_W, dec_b, scale, distance_weight):
    f32 = np.float32
    support = np.asarray(support, f32)
    query = np.asarray(query, f32)
    x = np.asarray(x, f32)

    # ---------- host: constant folding of weights ----------
    weff_f = (np.asarray(flat_W1, f32) @ np.asarray(flat_W2, f32)).astype(f32)
    weff_s = (np.asarray(score_W1, f32) @ np.asarray(score_W2, f32)).astype(f32)
    weff = np.concatenate([weff_f, weff_s], axis=1)  # [12544, 69]
    beff_f = (np.asarray(flat_b1, f32) @ np.asarray(flat_W2, f32) +
              np.asarray(flat_b2, f32)).astype(f32)
    beff_s = (np.asarray(score_b1, f32) @ np.asarray(score_W2, f32) +
              np.asarray(score_b2, f32)).astype(f32)
    beff = np.concatenate([beff_f, beff_s])[:, None].astype(f32)  # [69,1]

    # ---------- host: support branch (tiny) + loss_aux ----------
    sflat = support.reshape(WAY, -1)  # [5, 12544]
    sf = _leaky_np(sflat @ weff_f + beff_f)  # [5, 64]
    ss = _leaky_np(sflat @ weff_s + beff_s)  # [5, 5]

    def enc_conv_np(t, mu=None, var=None):
        y = np.einsum('nchw,oc->nohw', t, np.asarray(conv_W, f32),
                      optimize=True) + np.asarray(conv_b, f32)[None, :, None, None]
        if mu is None:
            mu = y.mean((0, 2, 3), keepdims=True)
            var = ((y - mu) ** 2).mean((0, 2, 3), keepdims=True)
        yn = (y - mu) / np.sqrt(var + BN_EPS)
        yn = yn * np.asarray(bn_gamma, f32)[None, :, None, None] + \
            np.asarray(bn_beta, f32)[None, :, None, None]
        return _leaky_np(yn), mu, var

    sc, _, _ = enc_conv_np(support)

    def auxrank_np(s):
        nrm = np.sqrt((s * s).sum(1, keepdims=True))
        s = s / np.maximum(nrm, 1e-12)
        sq = (s.reshape(WAY, -1)) ** 2
        tot = sq.sum(0)
        return (((tot * tot) - (sq * sq).sum(0)) * 0.5).mean()

    loss_aux = np.asarray(auxrank_np(sf) + auxrank_np(sc), f32)

    sfm = sf.mean(1)  # [5]
    scm = sc.mean((1, 2, 3))  # [5]
    ssm = ss.mean(1)  # [5]

    # ---------- host: global BN stats for the query conv branch ----------
    qf2 = query.transpose(0, 2, 3, 1).reshape(-1, C)  # [N*hw, C]
    yq = qf2 @ np.asarray(conv_W, f32).T  # [N*hw, 64]
    yq += np.asarray(conv_b, f32)[None, :]
    mu_q = yq.mean(0)  # [64]
    var_q = ((yq - mu_q[None, :]) ** 2).mean(0)  # [64]
    rstd = (1.0 / np.sqrt(var_q + BN_EPS)).astype(f32)
    bna = (np.asarray(bn_gamma, f32) * rstd)[:, None].astype(f32)  # scale on conv out (incl bias)
    # device computes yc (no conv bias); z = bna*yc + bnd
    bnd = ((np.asarray(conv_b, f32) - mu_q) * bna[:, 0] +
           np.asarray(bn_beta, f32))[:, None].astype(f32)

    # ---------- host: small scalars ----------
    wts = np.exp(np.exp(np.asarray(distance_weight, f32)))
    wts = (wts / wts.sum()).astype(f32)  # softmax(exp(dw))
    es = float(np.exp(np.asarray(scale, f32))[0])
    s_f = float(np.sqrt(wts[0]) / HID)
    s_c = float(np.sqrt(wts[1]) / (HID * HW))
    s_s = float(np.sqrt(wts[2]) / WAY)
    sqb = np.stack([-np.sqrt(wts[0]) * sfm,
                    -np.sqrt(wts[1]) * scm,
                    -np.sqrt(wts[2]) * ssm], axis=1).astype(f32)  # [5,3]

    # ---------- host: input marshalling (layout only) ----------
    qT = np.ascontiguousarray(query.reshape(BOX, FEAT).T)  # [12544, 4096]
    xT = np.ascontiguousarray(x.T)  # [1024, 4096]
    # weff rows feat=(cc*128+p)*49+hw -> sbuf [p, (hw*2+cc)*69+j]
    weffsw = np.ascontiguousarray(
        weff.reshape(2, 128, HW, NOUT).transpose(1, 2, 0, 3).reshape(128, 98 * NOUT))
    convwt = np.ascontiguousarray(
        np.asarray(conv_W, f32).T.reshape(2, 128, HID).transpose(1, 0, 2).reshape(128, 2 * HID))
    wcat = np.concatenate([np.asarray(bg_W, f32),
                           np.asarray(bbox_W, f32)], axis=1)  # [1024, 25]
    wcatsw = np.ascontiguousarray(
        wcat.reshape(8, 128, 25).transpose(1, 0, 2).reshape(128, 200))
    bcat = np.concatenate([np.asarray(bg_b, f32),
                           np.asarray(bbox_b, f32)])[:, None].astype(f32)
    rmask = np.zeros((NOUT, 2), f32)
    rmask[0:HID, 0] = 1.0
    rmask[HID:NOUT, 1] = 1.0
    sel = np.zeros((2, 10), f32)
    sel[0, 0:5] = 1.0
    sel[1, 5:10] = 1.0
    decw = np.asarray(dec_W, f32)
    decb = np.asarray(dec_b, f32)[:, None].astype(f32)

    consts = {"s_f": s_f, "s_c": s_c, "s_s": s_s, "es": es}
    key = tuple(sorted(consts.items()))
    if key not in _compiled:
        _compiled[key] = _build_program(consts)
    nc = _compiled[key]

    shared = {"weffsw": weffsw, "convwt": convwt, "wcatsw": wcatsw,
              "beff": beff, "bna": bna, "bnd": bnd, "bcat": bcat,
              "decw": decw, "decb": decb, "sqb": sqb, "rmask": rmask,
              "sel": sel}
    in_maps = []
    for i in range(NCORES):
        m = dict(shared)
        m["qT"] = np.ascontiguousarray(qT[:, i * B:(i + 1) * B])
        m["xT"] = np.ascontiguousarray(xT[:, i * B:(i + 1) * B])
        in_maps.append(m)

    global LAST, LAST_INMAPS, LAST_NC
    try:
        res = run_bass_kernel_spmd(nc, in_maps, core_ids=list(range(NCORES)),
                                   trace=TRACE)
    except (ImportError, ModuleNotFoundError):
        res = run_bass_kernel_spmd(nc, in_maps, core_ids=list(range(NCORES)))
    LAST = res
    LAST_INMAPS = in_maps
    LAST_NC = nc
    results = res.results

    scores = np.concatenate([r["scoresT"] for r in results], axis=1).T.copy()
    bbox = np.concatenate([r["bboxT"] for r in results], axis=1).T.copy()
    return scores.astype(f32), bbox.astype(f32), loss_aux
